# revision 18
# baseline (speedup 1.0000x reference)
"""Trainium2 Bass kernel for nn_BlockWithFFN (B=8192, S=128, D=6).

Data-parallel over 8 NeuronCores. The axon tunnel (~85ms fixed RPC cost,
~55MB/s each way) dominates wall time, so transfers are quantized hard:

  host:   LN1 in f32; ship z1 as 6-bit codes v = rint(z1*31.5/sqrt(5))+32
          (|z1| < sqrt(5) strictly), 4 codes plane-packed into 3 bytes
          (byte group j holds stream positions j+160r so the device
          unpack planes land contiguous), plus s1 as a +128-biased log-u8
          code. Dequant scale ALPHA = 2*sqrt(5)/63 folds into the
          attention consts (Ahat *= a^2, chat *= a, Avo *= a) and s1'.
  device: unpack 6-bit codes with ACT-rne floor emulation + stt mult/add
          (f32 lanes, u8 writeback), attention on raw-code z1 slabs,
          u2 = s1'*z1_r + o (LN2 is shift-invariant so the mean m1 never
          needs to exist on device), LN2 + FFN (b2 rides an aug ones row
          of the W2 matmul), returns delta = o + ff as 4-bit codes
          q = rne(delta/shat*7.5 + 7.5) with a per-token u8 log-scale
          shat (ceil-encoded via the ACT-rne bias so shat >= max|delta|;
          rec = 1/shat computed as Exp of the rounded code so host and
          device agree), two codes packed per byte with one stt.
  host:   unpack nibbles with a 256-entry scale table,
          out = x + (q - 7.5)*shat/7.5 in f32.

Runner: the jitted shard_map executable, device-resident weight consts,
and output donation buffers are cached across calls (run_bass_kernel_spmd
re-jits per call, which costs ~1s of XLA compile + NEFF reload; this is
the same bass2jax PJRT path minus the re-jit). NCHUNK=1: on this 1-CPU
host the axon loopback relay competes with python for the core, so
chunked quant/transfer overlap loses to its extra fixed per-transfer
costs (~45ms per put, ~70ms per fetch). Per-core h2d/exec/d2h already
pipeline naturally since each core's shard transfers independently.

If NCHUNK is ever raised: _quant_chunk reuses one scratch buffer, so a
chunk's device_put must be known complete (or copied) before the next
chunk's quant overwrites it.

On-chip layout: batch-major int8 DMA, ACT i8->bf16, PE-transpose
restructure to token slabs [128s, (b,d)], 4-batch pack transposes for
the PE, per-batch attention with fp32 PSUM accumulation, two ACT table
phases (ln/exp then gelu).
"""

import sys

sys.path.insert(0, "/opt/trn_rl_repo")

import contextlib
import hashlib

import numpy as np

import concourse.bass as bass
import concourse.mybir as mybir
import concourse.tile as tile
from concourse import bacc

F32 = mybir.dt.float32
BF16 = mybir.dt.bfloat16
I8 = mybir.dt.int8
AF = mybir.ActivationFunctionType
ALU = mybir.AluOpType
AX = mybir.AxisListType

D = 6
S = 128
B = 8192
NCORES = 8
NCHUNK = 1          # pipelined fn calls per kernel() invocation
EPS = 1e-5
PW = 4              # pack width (tile_position 32-alignment)
LZ = 32             # z-slab stride per batch: z(6) | ones | pad
RB = 8              # batches per attention round
GELU_FN = AF.Gelu

ZMAX = float(np.sqrt(5.0))       # strict bound on |z| for D=6 layernorm
ALPHA = 2.0 * ZMAX / 63.0        # 6-bit z dequant step (folded into consts)
# delta ships as 5-bit codes q = rne(delta/shat*15.5 + 15.5) with a
# per-token u8 log-scale shat >= max_d|delta| (ceil-encoded so codes never
# clip); three codes pack into two bytes: b0 = v0 + 32*(v1%8),
# b1 = v1//8 + 4*v2.
K2 = 255.0 / float(np.log(2.0 / 0.02))   # delta-scale log-code gain
LMIN = float(np.log(0.02))               # delta-scale log-code offset
CPK_OUT = 2 * S * D // 3 + S     # 512 packed delta bytes + 128 scale bytes
# log-u8 encoding of s1 (trailing S bytes of the z1 tensor), +128 biased:
# s in [sqrt(eps), 4.0] covers any N(0,1)-ish row
SMIN, SMAX_S = float(np.sqrt(EPS)), 4.0
KQ = 254.0 / (np.log(SMAX_S) - np.log(SMIN))
MIDQ = 0.5 * (np.log(SMAX_S) + np.log(SMIN))
D5 = D - 1                       # ship 5 of 6 z dims; sum(z)=0 gives the 6th
PKZ = 3 * (S * D5 // 4)          # 480 plane-packed 6-bit z bytes (4 in 3)
CPACK = PKZ + S                  # packed row: 480 z bytes | 128 s1 bytes

_CACHE = {}


# --------------------------------------------------------------------------
# host-side weight folding (int8 scale ALPHA folded in)
# --------------------------------------------------------------------------
def _fold_weights(ln1_w, ln1_b, wqkv, bqkv, wo, bo, ln2_w, ln2_b, w1, b1, w2, b2):
    f64 = np.float64
    (ln1_w, ln1_b, wqkv, bqkv, wo, bo, ln2_w, ln2_b, w1, b1, w2, b2) = [
        np.asarray(a, f64)
        for a in (ln1_w, ln1_b, wqkv, bqkv, wo, bo, ln2_w, ln2_b, w1, b1, w2, b2)
    ]
    Wq, Wk, Wv = wqkv[:, 0:D], wqkv[:, D : 2 * D], wqkv[:, 2 * D :]
    bq, bk, bv = bqkv[0:D], bqkv[D : 2 * D], bqkv[2 * D :]
    Dw = np.diag(ln1_w)
    Aq, cq = Dw @ Wq, ln1_b @ Wq + bq
    Ak, ck = Dw @ Wk, ln1_b @ Wk + bk
    Av, cv0 = Dw @ Wv, ln1_b @ Wv + bv
    sc = 1.0 / np.sqrt(D)
    return dict(
        Ahat=(Aq @ Ak.T) * sc * ALPHA * ALPHA,
        chat=(cq @ Ak.T) * sc * ALPHA,
        Avo=(Av @ wo) * ALPHA,
        cvo=cv0 @ wo + bo,
        W1z=np.diag(ln2_w) @ w1,
        c1=ln2_b @ w1 + b1,
        w2=w2,
        b2=b2,
    )


def _rep_const(mat, aug_row):
    """[128, 6] tile: rows 32c+d = mat[d, :], row 32c+6 = aug_row (c=0..3)."""
    t = np.zeros((128, D), np.float64)
    for c in range(PW):
        t[32 * c : 32 * c + D, :] = mat
        if aug_row is not None:
            t[32 * c + D, :] = aug_row
    return t.astype(np.float32)


def _blk_const(mat, aug_row):
    """[128, PW*D]: rows 32c+j, cols 6c+m hold mat[j, m]; row 32c+6 = aug."""
    t = np.zeros((128, PW * D), np.float64)
    for c in range(PW):
        t[32 * c : 32 * c + D, D * c : D * c + D] = mat
        if aug_row is not None:
            t[32 * c + D, D * c : D * c + D] = aug_row
    return t.astype(np.float32)


def _build_consts(fw):
    mask_kq = (np.arange(S)[:, None] <= np.arange(S)[None, :]).astype(np.float32)
    ident = np.eye(S, dtype=np.float32)
    ac = _rep_const(fw["Ahat"], fw["chat"])
    ac32 = np.zeros((128, 32), np.float32)
    ac32[:, 0:D] = ac
    acbig = np.zeros((128, 128), np.float32)
    for c in range(PW):
        acbig[32 * c : 32 * c + 32, 32 * c : 32 * c + 32] = ac32[
            32 * c : 32 * c + 32, :
        ]
    return dict(
        Ac=acbig,
        Avoc=_blk_const(fw["Avo"], fw["cvo"]),
        W1c=_blk_const(fw["W1z"], fw["c1"]),
        W2c=_blk_const(fw["w2"], fw["b2"]),
        maskkq=mask_kq,
        identm=ident,
    )


# --------------------------------------------------------------------------
# bass program
# --------------------------------------------------------------------------
def build_nc(bc, gb):
    """bc = batches per core per chunk, gb = batches per group."""
    assert gb % RB == 0 and RB % PW == 0 and bc % gb == 0
    nc = bacc.Bacc("TRN2", target_bir_lowering=False, debug=False)
    zq_d = nc.dram_tensor("zq", [bc, CPACK], I8, kind="ExternalInput")
    dq_d = nc.dram_tensor("dq", [bc, CPK_OUT], mybir.dt.uint8,
                          kind="ExternalOutput")
    c_d = {
        name: nc.dram_tensor(name, shape, F32, kind="ExternalInput")
        for name, shape in [
            ("Ac", [128, 128]),
            ("Avoc", [128, PW * D]),
            ("W1c", [128, PW * D]),
            ("W2c", [128, PW * D]),
            ("maskkq", [S, S]),
            ("identm", [S, S]),
        ]
    }
    with tile.TileContext(nc) as tc:
        _emit(tc, nc, zq_d, dq_d, c_d, bc, gb)
    nc.compile()
    return nc


def _emit_out_q(tc, nc, bigsb, work, identf, recb, out_v, g, gb, slab):
    """token slab [128, gb*D] f32 -> per-token scale shat (u8 log-code,
    ceil-encoded so shat >= max_d|delta|) + 4-bit codes rne(delta/shat*7.5
    + 7.5), two codes per byte -> DMA out [gb, 384+128]."""
    U8 = mybir.dt.uint8
    smax = bigsb.tile([128, gb], F32, tag="smax")
    nc.vector.reduce_max(
        smax,
        slab.rearrange("p (b d) -> p b d", d=D),
        axis=AX.X,
        apply_absolute_value=True,
    )
    lns = bigsb.tile([128, gb], F32, tag="lns")
    nc.scalar.activation(lns, smax, AF.Ln)
    ucode = bigsb.tile([128, gb], U8, tag="ucode")
    nc.scalar.activation(
        ucode, lns, AF.Copy, bias=float(0.5 - K2 * LMIN), scale=float(K2)
    )
    rec = bigsb.tile([128, gb], F32, tag="recs")
    nc.scalar.activation(
        rec, ucode, AF.Exp, bias=recb, scale=float(-1.0 / K2)
    )
    tq = bigsb.tile([128, gb * D], F32, tag="tqs")
    nc.vector.tensor_tensor(
        tq.rearrange("p (b d) -> p b d", d=D),
        slab.rearrange("p (b d) -> p b d", d=D),
        rec[:][:, :, None].broadcast_to([128, gb, D]),
        op=ALU.mult,
    )
    ucf = bigsb.tile([128, gb], F32, tag="ucf")
    nc.scalar.activation(ucf, ucode, AF.Copy)
    ot_ps = work.tile([128, 2048], F32, tag="work")
    for d in range(D):
        nc.tensor.matmul(
            ot_ps[0:gb, S * d : S * (d + 1)],
            tq.rearrange("p (b d) -> p d b", d=D)[:, d, :],
            identf,
            is_transpose=True,
        )
    nc.tensor.matmul(
        ot_ps[0:gb, S * D : S * (D + 1)], ucf, identf, is_transpose=True
    )
    vq = bigsb.tile([gb, S * D], U8, tag="vq")
    nc.scalar.activation(
        vq.rearrange("p (s d) -> p d s", d=D),
        ot_ps[0:gb, 0 : S * D].rearrange("p (d s) -> p d s", s=S),
        AF.Copy,
        bias=15.5,
        scale=15.5,
    )
    # clamp high side so an out-of-range code can't corrupt its pack-mates
    # (low side already saturates to 0 in the u8 cast)
    nc.vector.tensor_scalar(vq, vq, 31, None, op0=ALU.min)
    NG3 = S * D // 3  # 256 pack groups of 3 codes -> 2 bytes
    q3 = vq.rearrange("p (k j) -> p k j", j=3)
    pf1 = bigsb.tile([gb, NG3], U8, tag="pf1")
    nc.scalar.activation(pf1, q3[:, :, 1], AF.Copy, bias=-0.4375, scale=0.125)
    pm1 = bigsb.tile([gb, NG3], U8, tag="pm1")
    nc.vector.scalar_tensor_tensor(
        pm1, pf1, -8.0, q3[:, :, 1], op0=ALU.mult, op1=ALU.add
    )
    pk = bigsb.tile([gb, CPK_OUT], U8, tag="pk")
    pk2 = pk[:, 0 : 2 * NG3].rearrange("p (k i) -> p k i", i=2)
    nc.vector.scalar_tensor_tensor(
        pk2[:, :, 0], pm1, 32.0, q3[:, :, 0], op0=ALU.mult, op1=ALU.add
    )
    nc.vector.scalar_tensor_tensor(
        pk2[:, :, 1], q3[:, :, 2], 4.0, pf1, op0=ALU.mult, op1=ALU.add
    )
    nc.scalar.activation(
        pk[:, 2 * NG3 : CPK_OUT], ot_ps[0:gb, S * D : S * (D + 1)], AF.Copy
    )
    nc.sync.dma_start(out=out_v[g, :, :], in_=pk)


def _emit(tc, nc, zq_d, dq_d, c_d, bc, gb):
    ng = bc // gb
    nr = gb // RB
    ctx = contextlib.ExitStack()
    with ctx:
        singles = ctx.enter_context(tc.tile_pool(name="singles", bufs=1))
        cs = {}
        for name in ("Ac", "Avoc", "W1c", "W2c"):
            w = 128 if name == "Ac" else PW * D
            t = singles.tile([128, w], BF16, tag=name)
            nc.gpsimd.dma_start(out=t, in_=c_d[name][:, :])
            cs[name] = t
        maskb = singles.tile([S, S], BF16, tag="maskb")
        nc.gpsimd.dma_start(out=maskb, in_=c_d["maskkq"][:, :])
        identb = singles.tile([S, S], BF16, tag="identb")
        nc.gpsimd.dma_start(out=identb, in_=c_d["identm"][:, :])
        identf = singles.tile([S, S], F32, tag="identf")
        nc.sync.dma_start(out=identf, in_=c_d["identm"][:, :])
        epst = singles.tile([128, 1], F32, tag="epst")
        nc.vector.memset(epst, EPS)
        sdecb = singles.tile([128, 1], F32, tag="sdecb")
        nc.vector.memset(sdecb, float(MIDQ + np.log(ALPHA) - 128.0 / KQ))
        recb = singles.tile([128, 1], F32, tag="recb")
        nc.vector.memset(recb, float(-LMIN))

        delta_pool = ctx.enter_context(tc.tile_pool(name="deltas", bufs=ng))
        g_pool = ctx.enter_context(tc.tile_pool(name="gslabs", bufs=ng))
        bigsb = ctx.enter_context(tc.tile_pool(name="bigsb", bufs=2))
        u2pool = ctx.enter_context(tc.tile_pool(name="u2p", bufs=2))
        sxpool = ctx.enter_context(tc.tile_pool(name="sxp", bufs=2))
        s1pool = ctx.enter_context(tc.tile_pool(name="s1p", bufs=2))
        work = ctx.enter_context(tc.tile_pool(name="work", bufs=1, space="PSUM"))
        smps = ctx.enter_context(tc.tile_pool(name="smps", bufs=4, space="PSUM"))
        smsb = ctx.enter_context(tc.tile_pool(name="smsb", bufs=4))
        epool = ctx.enter_context(tc.tile_pool(name="epool", bufs=2))
        stpool = ctx.enter_context(tc.tile_pool(name="stpool", bufs=2))
        zpool = ctx.enter_context(tc.tile_pool(name="zpool", bufs=2))

        zq_v = zq_d.rearrange("(g b) c -> g b c", g=ng)
        out_v = dq_d.rearrange("(g b) c -> g b c", g=ng)

        delta_slabs, g_slabs = [], []

        def layernorm_z(src_slab, zslab_tag, rstd_tag):
            """token slab [128, gb*D] f32 -> z-slab bf16 (LZ-strided, aug ones)."""
            sum1 = stpool.tile([128, gb], F32, tag=rstd_tag + "s1")
            nc.vector.reduce_sum(
                sum1, src_slab.rearrange("p (b d) -> p b d", d=D), axis=AX.X
            )
            sq = bigsb.tile([128, gb * D], F32, tag="scratch")
            nc.vector.tensor_tensor(sq, src_slab, src_slab, op=ALU.mult)
            ssq = stpool.tile([128, gb], F32, tag=rstd_tag + "s2")
            nc.vector.reduce_sum(
                ssq, sq.rearrange("p (b d) -> p b d", d=D), axis=AX.X
            )
            mean = stpool.tile([128, gb], F32, tag=rstd_tag + "m")
            nc.vector.tensor_scalar_mul(mean, sum1, 1.0 / D)
            var = stpool.tile([128, gb], F32, tag=rstd_tag + "v")
            nc.vector.tensor_scalar_mul(var, ssq, 1.0 / D)
            msq = stpool.tile([128, gb], F32, tag=rstd_tag + "mq")
            nc.vector.tensor_tensor(msq, mean, mean, op=ALU.mult)
            nc.vector.tensor_tensor(var, var, msq, op=ALU.subtract)
            rstd = stpool.tile([128, gb], F32, tag=rstd_tag + "r")
            nc.scalar.activation(rstd, var, AF.Ln, bias=epst)
            nc.scalar.activation(rstd, rstd, AF.Exp, scale=-0.5)
            zslab = zpool.tile([128, gb * LZ], BF16, tag=zslab_tag)
            cen = bigsb.tile([128, gb * D], F32, tag="scratch2")
            nc.vector.tensor_tensor(
                cen.rearrange("p (b d) -> p b d", d=D),
                src_slab.rearrange("p (b d) -> p b d", d=D),
                mean[:][:, :, None].broadcast_to([128, gb, D]),
                op=ALU.subtract,
            )
            nc.vector.tensor_tensor(
                zslab[:, 0 : gb * LZ].rearrange("p (b l) -> p b l", l=LZ)[:, :, 0:D],
                cen.rearrange("p (b d) -> p b d", d=D),
                rstd[:][:, :, None].broadcast_to([128, gb, D]),
                op=ALU.mult,
            )
            nc.vector.memset(
                zslab[:, 0 : gb * LZ].rearrange("p (b l) -> p b l", l=LZ)[
                    :, :, D : D + 1
                ],
                1.0,
            )
            nc.vector.memset(
                zslab[:, 0 : gb * LZ].rearrange("p (b l) -> p b l", l=LZ)[
                    :, :, D + 1 : LZ
                ],
                0.0,
            )
            return zslab

        def ffn_matmul(zslab, wtile, out_cb, naug):
            """4 batches per full-width matmul: zT1 [128,128] (32-row groups)
            x block-diag wtile [128, 24] -> [128 tok, 24] = (b,d)-contiguous.
            Pad rows of zslab are zeroed so the full contraction is exact."""
            hb = min(64, gb)
            for h in range(gb // hb):
                g_ps = work.tile([128, 2048], F32, tag="work")
                for p in range(hb // PW):
                    pk = (h * hb) // PW + p
                    zp = smps.tile([128, 1024], BF16, tag="sps")
                    nc.tensor.matmul(
                        zp[:, 0:128],
                        zslab[:, 128 * pk : 128 * (pk + 1)],
                        identb,
                        is_transpose=True,
                    )
                    zT1 = smsb.tile([128, 128], BF16, tag="z2T")
                    nc.vector.tensor_copy(zT1, zp[:, 0:128])
                    nc.tensor.matmul(
                        g_ps[:, PW * D * p : PW * D * (p + 1)],
                        zT1,
                        wtile,
                    )
                out_cb(h, g_ps)

        # ============== PHASE 1: ln/exp table ==============
        for g in range(ng):
            U8 = mybir.dt.uint8
            zbm_i8 = bigsb.tile([gb, CPACK], I8, tag="zbm8")
            nc.sync.dma_start(out=zbm_i8, in_=zq_v[g, :, :])
            zbm_u8 = zbm_i8[:].bitcast(U8)
            # unpack 4x 6-bit codes from 3 bytes (plane layout: byte group j
            # holds stream positions {j, j+160, j+320, j+480}):
            #   b0 = v0 + 64*(v1%4); b1 = v1//4 + 16*(v2%16); b2 = v2//16 + 4*v3
            # floor-divs emulated with ACT rne; mod/recombine via stt (f32
            # lanes, u8 writeback).
            lanes = zbm_u8[:, 0:PKZ].rearrange("p (j i) -> p i j", i=3)
            NQ = S * D5 // 4  # 160 codes per plane
            f0 = bigsb.tile([gb, NQ], U8, tag="uf0")
            nc.scalar.activation(
                f0, lanes[:, 0, :], AF.Copy, bias=-0.4921875, scale=1.0 / 64
            )
            f1 = bigsb.tile([gb, NQ], U8, tag="uf1")
            nc.scalar.activation(
                f1, lanes[:, 1, :], AF.Copy, bias=-0.46875, scale=1.0 / 16
            )
            vq6 = bigsb.tile([gb, S * D5], U8, tag="vq6")
            nc.scalar.activation(
                vq6[:, 3 * NQ : 4 * NQ], lanes[:, 2, :], AF.Copy,
                bias=-0.375, scale=0.25,
            )
            m1t = bigsb.tile([gb, NQ], U8, tag="um1")
            nc.vector.scalar_tensor_tensor(
                m1t, f1, -16.0, lanes[:, 1, :], op0=ALU.mult, op1=ALU.add
            )
            m2t = bigsb.tile([gb, NQ], U8, tag="um2")
            nc.vector.scalar_tensor_tensor(
                m2t, vq6[:, 3 * NQ : 4 * NQ], -4.0, lanes[:, 2, :],
                op0=ALU.mult, op1=ALU.add,
            )
            nc.vector.scalar_tensor_tensor(
                vq6[:, 0:NQ], f0, -64.0, lanes[:, 0, :],
                op0=ALU.mult, op1=ALU.add,
            )
            nc.vector.scalar_tensor_tensor(
                vq6[:, NQ : 2 * NQ], m1t, 4.0, f0, op0=ALU.mult, op1=ALU.add
            )
            nc.vector.scalar_tensor_tensor(
                vq6[:, 2 * NQ : 3 * NQ], m2t, 16.0, f1,
                op0=ALU.mult, op1=ALU.add,
            )
            z_bm = bigsb.tile([gb, S * D5], BF16, tag="zbmh")
            nc.scalar.activation(z_bm, vq6, AF.Copy, bias=-32.0)
            s1_bm = s1pool.tile([gb, S], BF16, tag="s1bm")
            nc.scalar.activation(
                s1_bm,
                zbm_u8[:, PKZ:CPACK],
                AF.Exp,
                bias=sdecb,
                scale=float(1.0 / KQ),
            )

            # PE transposes: z (per shipped d) and s1 -> token-major
            zt_ps = smps.tile([128, 1024], BF16, tag="sps")
            for d in range(D5):
                nc.tensor.matmul(
                    zt_ps[:, gb * d : gb * (d + 1)],
                    z_bm.rearrange("p (s d) -> p d s", d=D5)[:, d, :],
                    identb[0:gb, 0:gb],
                    is_transpose=True,
                )
            s1t_ps = smps.tile([128, 1024], BF16, tag="sps")
            nc.tensor.matmul(
                s1t_ps[:, 0:gb], s1_bm, identb[0:gb, 0:gb], is_transpose=True
            )
            s1_tok = s1pool.tile([128, gb], F32, tag="s1tok")
            nc.vector.tensor_copy(s1_tok, s1t_ps[:, 0:gb])

            # z slab (LZ-strided, aug ones); dim 5 = -sum(dims 0..4)
            zslab = zpool.tile([128, gb * LZ], BF16, tag="z1")
            nc.vector.tensor_copy(
                zslab[:, 0 : gb * LZ].rearrange("p (b l) -> p l b", l=LZ)[:, 0:D5, :],
                zt_ps[:, 0 : gb * D5].rearrange("p (d b) -> p d b", b=gb),
            )
            z5t = s1pool.tile([128, gb], F32, tag="z5t")
            nc.vector.reduce_sum(
                z5t,
                zt_ps[:, 0 : gb * D5].rearrange("p (d b) -> p b d", b=gb),
                axis=AX.X,
            )
            nc.vector.tensor_scalar_mul(
                zslab[:, 0 : gb * LZ].rearrange("p (b l) -> p l b", l=LZ)[
                    :, D5 : D, :
                ][:, 0, :],
                z5t,
                -1.0,
            )
            nc.vector.memset(
                zslab[:, 0 : gb * LZ].rearrange("p (b l) -> p b l", l=LZ)[
                    :, :, D : D + 1
                ],
                1.0,
            )
            nc.vector.memset(
                zslab[:, 0 : gb * LZ].rearrange("p (b l) -> p b l", l=LZ)[
                    :, :, D + 1 : LZ
                ],
                0.0,
            )
            # sx = s1*z (f32, (b,d)-major) from the completed z slab
            sx_tok = sxpool.tile([128, gb * D], F32, tag="sx")
            nc.vector.tensor_tensor(
                sx_tok.rearrange("p (b d) -> p b d", d=D),
                zslab[:, 0 : gb * LZ].rearrange("p (b l) -> p b l", l=LZ)[
                    :, :, 0:D
                ],
                s1_tok[:][:, :, None].broadcast_to([128, gb, D]),
                op=ALU.mult,
            )

            u2_slab = u2pool.tile([128, gb * D], F32, tag="u2")
            delta_slab = delta_pool.tile([128, gb * D], F32, tag="delta")
            delta_slabs.append(delta_slab)

            for r in range(nr):
                b0 = r * RB
                # pack transposes -> zaugT (row-group 0, own bank)
                zpT = smps.tile([128, 1024], BF16, tag="sps")
                for p in range(RB // PW):
                    nc.tensor.matmul(
                        zpT[:, 128 * p : 128 * (p + 1)],
                        zslab[:, 128 * (b0 // PW + p) : 128 * (b0 // PW + p + 1)],
                        identb,
                        is_transpose=True,
                    )
                zT = smsb.tile([128, 256], BF16, tag="zT")
                nc.vector.tensor_copy(zT, zpT[:, 0 : 128 * (RB // PW)])

                # yhatT: block-diag AcBIG does all 4 batches of a pack at once
                yh_ps = smps.tile([128, 512], F32, tag="sps")
                for blk in range(RB // PW):
                    nc.tensor.matmul(
                        yh_ps[:, 128 * blk : 128 * (blk + 1)],
                        cs["Ac"],
                        zT[:, 128 * blk : 128 * (blk + 1)],
                    )
                yh = smsb.tile([128, 256], BF16, tag="yhsb")
                nc.vector.tensor_copy(yh, yh_ps[:, 0 : 32 * RB])

                # scores into work tile, bank c per row-group
                W = work.tile([128, 2048], F32, tag="work")
                for i in range(RB):
                    blk, c = i // PW, i % PW
                    nc.tensor.matmul(
                        W[:, 512 * c + 128 * blk : 512 * c + 128 * (blk + 1)],
                        zT[32 * c : 32 * c + D + 1, 128 * blk : 128 * (blk + 1)],
                        yh[32 * c : 32 * c + D + 1, 128 * blk : 128 * (blk + 1)],
                        tile_position=(32 * c, 0),
                    )
                # exp over the 4 score regions; eslab col = 256*c + 128*blk
                eslab = epool.tile([128, 1024], BF16, tag="E")
                sc_view = bass.AP(
                    tensor=W[:].tensor,
                    offset=W[:].offset,
                    ap=[list(W[:].ap[0]), [512, PW], [1, 256]],
                )
                nc.scalar.activation(eslab, sc_view, AF.Exp)
                nc.vector.tensor_tensor(
                    eslab.rearrange("p (i q) -> p i q", q=S),
                    eslab.rearrange("p (i q) -> p i q", q=S),
                    maskb[:][:, None, :].broadcast_to([S, RB, S]),
                    op=ALU.mult,
                )

                # v2: block-diag Avoc, one matmul per pack -> [128 tok, 24]
                v2_ps = smps.tile([128, 512], F32, tag="sps")
                for blk in range(RB // PW):
                    nc.tensor.matmul(
                        v2_ps[:, PW * D * blk : PW * D * (blk + 1)],
                        zT[:, 128 * blk : 128 * (blk + 1)],
                        cs["Avoc"],
                    )
                v2sb = smsb.tile([128, 8 * RB], BF16, tag="v2sb")
                nc.vector.tensor_copy(
                    v2sb.rearrange("p (c k l) -> p c k l", c=PW, k=2)[:, :, :, 0:D],
                    v2_ps[:, 0 : 2 * PW * D].rearrange(
                        "p (k c d) -> p c k d", c=PW, d=D
                    ),
                )
                nc.vector.memset(
                    v2sb.rearrange("p (j l) -> p j l", l=8)[:, :, D : D + 1], 1.0
                )

                # attn @ v2aug -> av slots (rg0, bank c per batch)
                for i in range(RB):
                    blk, c = i // PW, i % PW
                    j = 2 * c + blk
                    nc.tensor.matmul(
                        W[:, 512 * c + 320 + 16 * blk : 512 * c + 320 + 16 * blk + D + 1],
                        eslab[:, 256 * c + 128 * blk : 256 * c + 128 * (blk + 1)],
                        v2sb[:, 8 * j : 8 * j + D + 1],
                    )
                rec = smsb.tile([128, RB], F32, tag="rec")
                den_view = bass.AP(
                    tensor=W[:].tensor,
                    offset=W[:].offset + 320 + D,
                    ap=[list(W[:].ap[0]), [512, PW], [16, 2], [1, 1]],
                )
                nc.vector.reciprocal(
                    rec.rearrange("p (c k) -> p c k", c=PW)[:, :, :, None], den_view
                )
                t1 = smsb.tile([128, RB * D], F32, tag="t1")
                av_view = bass.AP(
                    tensor=W[:].tensor,
                    offset=W[:].offset + 320,
                    ap=[list(W[:].ap[0]), [512, PW], [16, 2], [1, D]],
                )
                nc.vector.tensor_tensor(
                    t1.rearrange("p (c k d) -> p c k d", c=PW, k=2),
                    av_view,
                    rec.rearrange("p (c k) -> p c k", c=PW)[:, :, :, None].broadcast_to(
                        [128, PW, 2, D]
                    ),
                    op=ALU.mult,
                )
                # u2[b0 + 4*blk + c] = t1[c, blk] + s1*z1[...]
                u2_out = bass.AP(
                    tensor=u2_slab[:].tensor,
                    offset=u2_slab[:].offset + D * b0,
                    ap=[list(u2_slab[:].ap[0]), [D, PW], [D * PW, 2], [1, D]],
                )
                sx_in = bass.AP(
                    tensor=sx_tok[:].tensor,
                    offset=sx_tok[:].offset + D * b0,
                    ap=[list(sx_tok[:].ap[0]), [D, PW], [D * PW, 2], [1, D]],
                )
                nc.vector.tensor_tensor(
                    u2_out,
                    t1.rearrange("p (c k d) -> p c k d", c=PW, k=2),
                    sx_in,
                    op=ALU.add,
                )

            # delta = o = u2 - sx (ff added in phase 2)
            nc.vector.tensor_tensor(
                delta_slab[:], u2_slab[:], sx_tok[:], op=ALU.subtract
            )

            # LN2 + W1 for whole group
            z2slab = layernorm_z(u2_slab, "z2", "r2")
            g_slab = g_pool.tile([128, gb * D], BF16, tag="g")
            g_slabs.append(g_slab)
            hb0 = min(64, gb)

            def g_out(h, g_ps, g_slab=g_slab, hb0=hb0):
                nc.vector.tensor_copy(
                    g_slab[:, D * hb0 * h : D * hb0 * (h + 1)],
                    g_ps[:, 0 : D * hb0],
                )

            ffn_matmul(z2slab, cs["W1c"], g_out, D + 1)

        # ============== PHASE 2: gelu table ==============
        tc.no_sync_barrier()
        for g in range(ng):
            g_slab, delta_slab = g_slabs[g], delta_slabs[g]
            gl = zpool.tile([128, gb * LZ], BF16, tag="gl")
            nc.scalar.activation(
                gl[:, 0 : gb * LZ].rearrange("p (b l) -> p b l", l=LZ)[:, :, 0:D],
                g_slab.rearrange("p (b d) -> p b d", d=D),
                GELU_FN,
            )
            nc.vector.memset(
                gl[:, 0 : gb * LZ].rearrange("p (b l) -> p b l", l=LZ)[
                    :, :, D : D + 1
                ],
                1.0,
            )
            nc.vector.memset(
                gl[:, 0 : gb * LZ].rearrange("p (b l) -> p b l", l=LZ)[
                    :, :, D + 1 : LZ
                ],
                0.0,
            )
            out_slab = bigsb.tile([128, gb * D], F32, tag="outslab")
            hb0 = min(64, gb)

            def f_out(h, f_ps, out_slab=out_slab, delta_slab=delta_slab, hb0=hb0):
                nc.vector.tensor_tensor(
                    out_slab[:, D * hb0 * h : D * hb0 * (h + 1)],
                    f_ps[:, 0 : D * hb0],
                    delta_slab[:, D * hb0 * h : D * hb0 * (h + 1)],
                    op=ALU.add,
                )

            ffn_matmul(gl, cs["W2c"], f_out, D + 1)
            _emit_out_q(tc, nc, bigsb, work, identf, recb, out_v, g, gb,
                        out_slab)


# --------------------------------------------------------------------------
# cached jit runner (replicates bass2jax.run_bass_via_pjrt, reusable)
# --------------------------------------------------------------------------
def _make_runner(nc, n_cores):
    import jax
    from jax.experimental.shard_map import shard_map
    from jax.sharding import Mesh, PartitionSpec
    from concourse.bass2jax import (
        _bass_exec_p,
        install_neuronx_cc_hook,
        partition_id_tensor,
    )

    install_neuronx_cc_hook()
    assert nc.dbg_addr is None
    pname = nc.partition_id_tensor.name if nc.partition_id_tensor else None
    in_names, out_names, out_avals = [], [], []
    for alloc in nc.m.functions[0].allocations:
        if not isinstance(alloc, mybir.MemoryLocationSet):
            continue
        name = alloc.memorylocations[0].name
        if alloc.kind == "ExternalInput":
            if name != pname:
                in_names.append(name)
        elif alloc.kind == "ExternalOutput":
            out_names.append(name)
            out_avals.append(
                jax.core.ShapedArray(
                    tuple(alloc.tensor_shape), mybir.dt.np(alloc.dtype)
                )
            )
    n_params = len(in_names)
    n_outs = len(out_names)
    all_names = in_names + out_names + ([pname] if pname else [])

    def _body(*args):
        ops = list(args)
        if pname:
            ops.append(partition_id_tensor())
        return tuple(
            _bass_exec_p.bind(
                *ops,
                out_avals=tuple(out_avals),
                in_names=tuple(all_names),
                out_names=tuple(out_names),
                lowering_input_output_aliases=(),
                sim_require_finite=True,
                sim_require_nnan=True,
                nc=nc,
            )
        )

    devices = jax.devices()[:n_cores]
    mesh = Mesh(np.asarray(devices), ("core",))
    fn = jax.jit(
        shard_map(
            _body,
            mesh=mesh,
            in_specs=(PartitionSpec("core"),) * (n_params + n_outs),
            out_specs=(PartitionSpec("core"),) * n_outs,
            check_rep=False,
        ),
        donate_argnums=tuple(range(n_params, n_params + n_outs)),
        keep_unused=True,
    )
    return dict(fn=fn, in_names=in_names, out_names=out_names,
                out_avals=out_avals, mesh=mesh)


# --------------------------------------------------------------------------
# host quantization
# --------------------------------------------------------------------------
_CSRC = r"""
#include <math.h>
#include <stdint.h>
void quant(const float* x, int8_t* bufi, long nb) {
  const float ZI = 31.5f / 2.23606797749978969f;   /* 1/step, 6-bit */
  const float KQ = 35.558625f;
  const float MIDQ = -2.18538250f;
  const float L2 = 0.69314718056f;
  for (long b = 0; b < nb; b++) {
    const float* xb = x + b * 768;
    uint8_t* zb = (uint8_t*)bufi + b * 608;
    uint8_t* sb = zb + 480;
    uint8_t vs[640];
    for (int t = 0; t < 128; t++) {
      const float* xt = xb + t * 6;
      float m = (xt[0]+xt[1]+xt[2]+xt[3]+xt[4]+xt[5]) * (1.0f/6.0f);
      float d0=xt[0]-m, d1=xt[1]-m, d2=xt[2]-m;
      float d3=xt[3]-m, d4=xt[4]-m, d5=xt[5]-m;
      float v = (d0*d0+d1*d1+d2*d2+d3*d3+d4*d4+d5*d5) * (1.0f/6.0f);
      float r = ZI / sqrtf(v + 1e-5f);
      uint8_t* vt = vs + t * 5;
      float c0 = rintf(d0*r), c1 = rintf(d1*r), c2 = rintf(d2*r);
      float c3 = rintf(d3*r), c4 = rintf(d4*r);
      if (c0 > 31.f) c0 = 31.f; if (c1 > 31.f) c1 = 31.f;
      if (c2 > 31.f) c2 = 31.f; if (c3 > 31.f) c3 = 31.f;
      if (c4 > 31.f) c4 = 31.f;
      vt[0]=(uint8_t)(c0+32.f); vt[1]=(uint8_t)(c1+32.f);
      vt[2]=(uint8_t)(c2+32.f); vt[3]=(uint8_t)(c3+32.f);
      vt[4]=(uint8_t)(c4+32.f);
      union { float f; uint32_t i; } uf; uf.f = v + 1e-5f;
      float e = (float)((int)(uf.i >> 23) - 127);
      uf.i = (uf.i & 0x007fffffu) | 0x3f800000u;
      float mm = uf.f;
      float zz = (mm - 1.0f) / (mm + 1.0f);
      float z2 = zz * zz;
      float lnv = e * L2 + 2.0f*zz*(1.0f + z2*(0.33333333f + z2*0.2f));
      float w = KQ * (0.5f * lnv - MIDQ) + 128.0f;
      if (w > 255.0f) w = 255.0f;
      if (w < 1.0f) w = 1.0f;
      sb[t] = (uint8_t)rintf(w);
    }
    /* plane pack: byte group j holds stream positions j, j+160, j+320,
       j+480: b0 = v0 + 64*(v1%4); b1 = v1/4 + 16*(v2%16); b2 = v2/16 + 4*v3 */
    for (int j = 0; j < 160; j++) {
      uint8_t v0 = vs[j], v1 = vs[j+160], v2 = vs[j+320], v3 = vs[j+480];
      zb[3*j]   = (uint8_t)(v0 | ((v1 & 3u) << 6));
      zb[3*j+1] = (uint8_t)((v1 >> 2) | ((v2 & 15u) << 4));
      zb[3*j+2] = (uint8_t)((v2 >> 4) | (v3 << 2));
    }
  }
}
void unpack_add(const unsigned char* pk, const float* x, float* out,
                long nb) {
  const float K2 = 55.36999f;      /* 255/ln(2.0/0.02) */
  const float LMIN = -3.91202301f; /* ln(0.02) */
  static float tbl[256]; static int init = 0;
  if (!init) {
    for (int u = 0; u < 256; u++)
      tbl[u] = expf((float)u * (1.0f/K2) + LMIN) * (1.0f/15.5f);
    init = 1;
  }
  for (long b = 0; b < nb; b++) {
    const unsigned char* pb = pk + b * 640;
    const unsigned char* sb = pb + 512;
    const float* xb = x + b * 768;
    float* ob = out + b * 768;
    for (int t = 0; t < 128; t++) {
      float st = tbl[sb[t]];
      const unsigned char* p4 = pb + t * 4;
      /* two groups of 3 codes from 2 bytes each:
         v0 = b0 % 32; v1 = b0/32 + 8*(b1 % 4); v2 = b1/4 */
      int b0 = p4[0], b1 = p4[1], b2 = p4[2], b3 = p4[3];
      ob[t*6+0] = xb[t*6+0] + ((float)(b0 & 31) - 15.5f) * st;
      ob[t*6+1] = xb[t*6+1] + ((float)((b0 >> 5) | ((b1 & 3) << 3)) - 15.5f) * st;
      ob[t*6+2] = xb[t*6+2] + ((float)(b1 >> 2) - 15.5f) * st;
      ob[t*6+3] = xb[t*6+3] + ((float)(b2 & 31) - 15.5f) * st;
      ob[t*6+4] = xb[t*6+4] + ((float)((b2 >> 5) | ((b3 & 3) << 3)) - 15.5f) * st;
      ob[t*6+5] = xb[t*6+5] + ((float)(b3 >> 2) - 15.5f) * st;
    }
  }
}
"""


def _get_clib():
    """Fused single-pass C quantizer (numpy path is ~4x slower on the
    1-CPU host). Any failure falls back to the numpy implementation."""
    if "clib" in _CACHE:
        return _CACHE["clib"]
    lib = None
    try:
        import ctypes
        import subprocess
        import tempfile
        import os

        d = tempfile.mkdtemp()
        src, so = os.path.join(d, "q.c"), os.path.join(d, "q.so")
        with open(src, "w") as f:
            f.write(_CSRC)
        subprocess.run(
            ["gcc", "-O3", "-march=native", "-shared", "-fPIC",
             "-o", so, src, "-lm"],
            check=True, capture_output=True, timeout=120,
        )
        lib = ctypes.CDLL(so)
        lib.quant.argtypes = [
            ctypes.POINTER(ctypes.c_float),
            ctypes.POINTER(ctypes.c_int8),
            ctypes.c_long,
        ]
        lib.quant.restype = None
        lib.unpack_add.argtypes = [
            ctypes.POINTER(ctypes.c_uint8),
            ctypes.POINTER(ctypes.c_float),
            ctypes.POINTER(ctypes.c_float),
            ctypes.c_long,
        ]
        lib.unpack_add.restype = None
    except Exception:
        lib = None
    _CACHE["clib"] = lib
    return lib


_ONES6 = np.ones(D, np.float32)


def _scratch(key, shape, dtype):
    buf = _CACHE.get(key)
    if buf is None or buf.shape != shape or buf.dtype != dtype:
        buf = _CACHE[key] = np.empty(shape, dtype)
    return buf


def _quant_chunk(xc):
    """xc [C, n, S, D] f32 contiguous -> packed u8 [C, n, CPACK] (int8 view):
    cols 0:480 = plane-packed 6-bit z codes (dims 0:5), cols 480:608 = s1
    log-u8 codes (+128 biased)."""
    lib = _get_clib()
    if lib is not None:
        import ctypes

        buf = _scratch("q_buf", xc.shape[:2] + (CPACK,), np.int8)
        xr = xc.reshape(-1, S * D)
        if not xr.flags.c_contiguous:
            xr = np.ascontiguousarray(xr)
        lib.quant(
            xr.ctypes.data_as(ctypes.POINTER(ctypes.c_float)),
            buf.ctypes.data_as(ctypes.POINTER(ctypes.c_int8)),
            xr.shape[0],
        )
        return buf
    m = (xc.reshape(-1, D) @ _ONES6).reshape(xc.shape[:-1] + (1,))
    m *= 1.0 / D
    d0 = _scratch("q_d0", xc.shape, np.float32)
    np.subtract(xc, m, out=d0)
    v = np.einsum("cnsd,cnsd->cns", d0, d0)
    s = np.sqrt(v * (1.0 / D) + EPS, dtype=np.float32)
    buf = _scratch("q_buf", d0.shape[:2] + (CPACK,), np.int8)
    bufu = buf.view(np.uint8)
    d0 *= ((31.5 / ZMAX) / s)[..., None]
    np.rint(d0, out=d0)
    np.clip(d0, -32.0, 31.0, out=d0)
    vs = (d0[..., 0:D5] + 32.0).astype(np.uint8).reshape(
        d0.shape[:2] + (S * D5,)
    )
    # plane pack 4->3 (byte group j <- stream positions j+160r)
    v0, v1 = vs[..., 0:160], vs[..., 160:320]
    v2, v3 = vs[..., 320:480], vs[..., 480:640]
    pk = bufu[..., 0:PKZ].reshape(bufu.shape[:2] + (160, 3))
    pk[..., 0] = v0 | ((v1 & 3) << 6)
    pk[..., 1] = (v1 >> 2) | ((v2 & 15) << 4)
    pk[..., 2] = (v2 >> 4) | (v3 << 2)
    w = np.log(s)
    w -= MIDQ
    w *= KQ
    w += 128.0
    np.rint(w, out=w)
    np.clip(w, 1.0, 255.0, out=w)
    bufu[..., PKZ:] = w
    return buf


# --------------------------------------------------------------------------
# public entry point
# --------------------------------------------------------------------------
def kernel(**inputs):
    import jax
    import jax.numpy as jnp
    from jax.sharding import NamedSharding, PartitionSpec

    x = np.asarray(inputs["x"], np.float32)
    assert x.shape == (B, S, D)
    bc_chunk = B // NCORES // NCHUNK

    if "nc" not in _CACHE:
        _CACHE["nc"] = build_nc(bc_chunk, 128)
        _CACHE["runner"] = _make_runner(_CACHE["nc"], NCORES)
    run = _CACHE["runner"]
    fn, mesh = run["fn"], run["mesh"]
    sh = NamedSharding(mesh, PartitionSpec("core"))

    # device-resident weight consts, keyed by weight bytes
    wnames = ("ln1_w", "ln1_b", "wqkv", "bqkv", "wo", "bo",
              "ln2_w", "ln2_b", "w1", "b1", "w2", "b2")
    wkey = hashlib.md5(
        b"".join(np.ascontiguousarray(np.asarray(inputs[n], np.float32)).tobytes()
                 for n in wnames)
    ).hexdigest()
    if _CACHE.get("wkey") != wkey:
        fw = _fold_weights(*[inputs[n] for n in wnames])
        consts = _build_consts(fw)
        dev_consts = {}
        for name, c in consts.items():
            c = np.ascontiguousarray(c, np.float32)
            g = np.broadcast_to(c, (NCORES,) + c.shape).reshape(
                (NCORES * c.shape[0],) + c.shape[1:]
            )
            dev_consts[name] = jax.device_put(np.ascontiguousarray(g), sh)
        for d in dev_consts.values():
            d.block_until_ready()
        _CACHE["wkey"] = wkey
        _CACHE["consts"] = dev_consts
    dev_consts = _CACHE["consts"]

    # donation buffers for the int8 outputs (kernel writes every element,
    # so any right-shaped buffer works; reuse previous outputs)
    if "donate" not in _CACHE:
        zfn = jax.jit(
            lambda: jnp.zeros((NCORES * bc_chunk, CPK_OUT), jnp.uint8),
            out_shardings=sh,
        )
        _CACHE["donate"] = [zfn() for _ in range(NCHUNK)]

    # chunk c covers contiguous batches [c*B/NCHUNK, (c+1)*B/NCHUNK); inside
    # a chunk, core k gets the k-th contiguous slice. Views stay contiguous.
    xch = x.reshape(NCHUNK, NCORES, B // NCHUNK // NCORES, S, D)

    # pipelined chunks: quant c -> h2d c (async) -> dispatch fn c; the next
    # chunk's host quant overlaps the previous chunk's h2d + exec.
    pending = []
    for c in range(NCHUNK):
        zi = _quant_chunk(xch[c])
        dz = jax.device_put(zi.reshape(-1, CPACK), sh)
        args = []
        for name in run["in_names"]:
            if name == "zq":
                args.append(dz)
            else:
                args.append(dev_consts[name])
        outs = fn(*args, _CACHE["donate"][c])
        pending.append(outs[0])

    out = np.empty((B, S, D), np.float32)
    och = out.reshape(NCHUNK, B // NCHUNK, S, D)
    xf = x.reshape(NCHUNK, B // NCHUNK, S, D)
    nck = B // NCHUNK
    clib = _get_clib()
    for c in range(NCHUNK):
        pk = np.asarray(pending[c]).reshape(nck, CPK_OUT)
        if clib is not None:
            import ctypes

            if not pk.flags.c_contiguous:
                pk = np.ascontiguousarray(pk)
            clib.unpack_add(
                pk.ctypes.data_as(ctypes.POINTER(ctypes.c_uint8)),
                xf[c].ctypes.data_as(ctypes.POINTER(ctypes.c_float)),
                och[c].ctypes.data_as(ctypes.POINTER(ctypes.c_float)),
                nck,
            )
            continue
        # numpy fallback: per-token scale table + 5-bit codes (3 in 2 bytes)
        utbl = _CACHE.get("utbl")
        if utbl is None:
            utbl = np.exp(
                np.arange(256, dtype=np.float32) / np.float32(K2)
                + np.float32(LMIN)
            ) * np.float32(1.0 / 15.5)
            _CACHE["utbl"] = utbl
        ng3 = 2 * S * D // 3                    # 512 packed bytes
        pb = pk[:, 0:ng3].reshape(nck, ng3 // 2, 2)
        b0, b1 = pb[..., 0], pb[..., 1]
        st = utbl[pk[:, ng3:]]                  # [nck, 128]
        v = np.empty((nck, S * D // 3, 3), np.float32)
        v[..., 0] = b0 & 31
        v[..., 1] = (b0 >> 5) | ((b1 & 3) << 3)
        v[..., 2] = b1 >> 2
        v = v.reshape(nck, S * D)
        v -= np.float32(15.5)
        v *= np.repeat(st, D, axis=1)
        och[c] = xf[c] + v.reshape(nck, S, D)
    _CACHE["donate"] = pending
    return out

